# revision 1
# baseline (speedup 1.0000x reference)
"""Trainium2 Bass kernel for a single-head causal self-attention variant.

Reference semantics (B=4, S=2048, D=1024):
    q = x @ wq.T ; k = x @ wk.T ; v = x @ wv.T
    scores = q @ k.T / sqrt(D)          # [B, S, S]
    a = softmax(scores, axis=-2)        # softmax over the QUERY axis, per key column
    a = triu(a)                         # keep q <= k, applied AFTER softmax
    out = a.T @ v                       # out row i = sum_{q<=i} a[q,i] * v[q]

Key algebraic folds (single head):
  * scores = x @ (wq.T @ wk) @ x.T, so wq/wk fold into one matrix
    MT = (wk.T @ wq)/sqrt(D) on the host -> no Q projection on device.
  * softmax needs no max subtraction here (scores are O(1) by construction),
    so a column is exp(s) / colsum, and the normalization can be applied to
    the output rows at the very end: out[k] = (sum_q Emask[q,k] v[q]) / colsum[k].

Sharding (8 cores): core = (batch b = core//2, half h = core%2). Each core owns
the interleaved local k-chunks kc_global = 2j+h, j=0..7 (128 columns each) of
its batch; the interleaving balances the triangular A^T V work between the two
halves. Softmax denominators need all 2048 q per column, so each core computes
scores/exp for all q in its 1024 columns; A^T V skips blocks that the causal
mask zeroes entirely.

All matmuls run as float32r (fp32 data, fp22 multiply) with 512-wide free
dims, which streams at 1 column/cycle on the PE like bf16 (fp32r matmuls are
self-loading, so each pays its 128-column weight load; measured ~260 us/core
steady-state vs a ~222 us PE floor). Column sums accumulate in two PSUM banks
across all 16 q-chunks, emitted after each group's A^T V matmuls so the
in-order PE stream never head-of-line blocks on an ACT exp. The device returns
unnormalized U plus the column-sum vector (cso); the O(S*D) elementwise
divide happens in gather() on the host, which lets each finished 128-row
output block DMA out mid-kernel instead of serializing a normalization tail.
A bf16 mode exists (use_bf16) but measured only ~8% faster at 16x worse
error, so fp32r is the default. End-to-end rel-l2 error ~2e-4 vs the fp32
reference.
"""

import numpy as np

B, S, D = 4, 2048, 1024
P = 128
SK = 1024          # k columns per core
KD = D // P        # 8 contraction chunks
NJ = SK // P       # 8 local k chunks
NG = 4             # q groups of 512
NQL = 4            # 128-row q chunks per group
NCORES = 8

_cache = {}
_ABLATE = set()  # test-only: {"phase3","av","cs","exp_ident","phasek"}


def _build_module(reps=1, accum=False, use_bf16=False):
    import concourse.bacc as bacc
    import concourse.tile as tile
    from concourse import mybir

    f32 = mybir.dt.float32
    f32r = mybir.dt.bfloat16 if use_bf16 else mybir.dt.float32r
    dmadt = f32r  # DRAM input dtype for matmul operands
    Exp = mybir.ActivationFunctionType.Exp

    nc = bacc.Bacc("TRN2", target_bir_lowering=False, debug=False,
                   num_devices=NCORES)

    xT = nc.dram_tensor("xT", [D, S], dmadt, kind="ExternalInput").ap()
    xtk = nc.dram_tensor("xtk", [D, SK], dmadt, kind="ExternalInput").ap()
    mt = nc.dram_tensor("mt", [D, D], dmadt, kind="ExternalInput").ap()
    wvT = nc.dram_tensor("wvT", [D, D], dmadt, kind="ExternalInput").ap()
    mask0 = nc.dram_tensor("mask0", [P, P], dmadt, kind="ExternalInput").ap()
    mask1 = nc.dram_tensor("mask1", [P, P], dmadt, kind="ExternalInput").ap()
    onesd = nc.dram_tensor("onesd", [P, 1], dmadt, kind="ExternalInput").ap()
    out = nc.dram_tensor("out", [SK, D], f32, kind="ExternalOutput").ap()
    cso = nc.dram_tensor("cso", [1, SK], f32, kind="ExternalOutput").ap()

    def mm(ps, lhsT, rhs, start, stop):
        nc.tensor.matmul(ps, lhsT, rhs, start=start, stop=stop)

    with tile.TileContext(nc) as tc:
        from contextlib import ExitStack
        for _rep in range(reps):
          with ExitStack() as ctx:
            persist = ctx.enter_context(tc.tile_pool(name="persist", bufs=1))
            psum = ctx.enter_context(tc.tile_pool(name="psum", bufs=2, space="PSUM"))

            ones_t = persist.tile([P, 1], f32r, tag="ones")
            nc.sync.dma_start(ones_t, onesd if use_bf16 else onesd.bitcast(f32r))
            m0_t = persist.tile([P, P], f32r, tag="m0")
            nc.sync.dma_start(m0_t, mask0 if use_bf16 else mask0.bitcast(f32r))
            m1_t = persist.tile([P, P], f32r, tag="m1")
            nc.sync.dma_start(m1_t, mask1 if use_bf16 else mask1.bitcast(f32r))

            wv_t = persist.tile([P, KD, D], f32r, tag="wv")
            for c in range(KD):
                nc.sync.dma_start(wv_t[:, c, :], (wvT if use_bf16 else wvT.bitcast(f32r))[c * P:(c + 1) * P, :])

            km_t = persist.tile([P, KD, SK], f32r, tag="km")
            u = [persist.tile([P, D], f32, tag=f"u{j}", name=f"u{j}")
                 for j in range(NJ)]

            # ---- phase K: KM[dq, k] = sum_dk M[dq,dk] * x_k^T[dk, k] ----
            with tc.tile_pool(name="pk", bufs=1) as pk:
                mt_t = pk.tile([P, KD, D], f32r, tag="mt")
                xtk_t = pk.tile([P, KD, SK], f32r, tag="xtk")
                for c in range(KD):
                    nc.sync.dma_start(mt_t[:, c, :], (mt if use_bf16 else mt.bitcast(f32r))[c * P:(c + 1) * P, :])
                    nc.sync.dma_start(xtk_t[:, c, :], (xtk if use_bf16 else xtk.bitcast(f32r))[c * P:(c + 1) * P, :])
                for dq in range(0 if "phasek" in _ABLATE else KD):
                    for kf in range(2):
                        ps = psum.tile([P, 512], f32, tag="ps_mm", name="ps_km", bufs=5)
                        for c in range(KD):
                            mm(ps, mt_t[:, c, dq * P:(dq + 1) * P],
                               xtk_t[:, c, kf * 512:(kf + 1) * 512],
                               start=(c == 0), stop=(c == KD - 1))
                        nc.vector.tensor_copy(km_t[:, dq, kf * 512:(kf + 1) * 512], ps)

            # ---- phase 2: stream q in 4 groups of 512 ----
            cs_ps = [psum.tile([1, 512], f32, tag=f"ps_cs{kf}", name=f"ps_cs{kf}",
                               bufs=1) for kf in range(2)]
            qgp = ctx.enter_context(tc.tile_pool(name="qgp", bufs=2))
            vegp = ctx.enter_context(tc.tile_pool(name="vegp", bufs=2))
            for g in range(NG):
                xg = qgp.tile([P, KD, 512], f32r, tag="xg", name=f"xg{g}")
                for c in range(KD):
                    nc.sync.dma_start(
                        xg[:, c, :],
                        (xT if use_bf16 else xT.bitcast(f32r))[c * P:(c + 1) * P, g * 512:(g + 1) * 512])
                eg, vg = [], []
                for ql in range(NQL):
                    # V[q, dv] for this 128-row q chunk
                    vt = vegp.tile([P, D], f32r, tag=f"v{ql}", name=f"v{g}_{ql}")
                    for dv in range(2):
                        ps = psum.tile([P, 512], f32, tag="ps_mm", name="ps_v", bufs=5)
                        for c in range(KD):
                            mm(ps, xg[:, c, ql * P:(ql + 1) * P],
                               wv_t[:, c, dv * 512:(dv + 1) * 512],
                               start=(c == 0), stop=(c == KD - 1))
                        nc.vector.tensor_copy(vt[:, dv * 512:(dv + 1) * 512], ps)
                    vg.append(vt)
                    # E[q, k] = exp(scores) for this q chunk x all local k
                    et = vegp.tile([P, SK], f32r, tag=f"e{ql}", name=f"e{g}_{ql}")
                    for kf in range(2):
                        ps = psum.tile([P, 512], f32, tag="ps_mm", name="ps_e", bufs=5)
                        for c in range(KD):
                            mm(ps, xg[:, c, ql * P:(ql + 1) * P],
                               km_t[:, c, kf * 512:(kf + 1) * 512],
                               start=(c == 0), stop=(c == KD - 1))
                        nc.scalar.activation(et[:, kf * 512:(kf + 1) * 512], ps, Exp)
                    eg.append(et)
                # causal mask: the j == qc//2 block is multiplied into a
                # separate tile (keeps eg read-only, so colsum and AV don't
                # serialize on a WAR hazard); blocks j > qc//2 are all-ones,
                # blocks j < qc//2 are never read by AV.
                emask = []
                for ql in range(NQL):
                    qc = g * NQL + ql
                    jm = qc // 2
                    mk = m0_t if qc % 2 == 0 else m1_t
                    em = vegp.tile([P, P], f32r, tag=f"em{ql}", name=f"em{g}_{ql}")
                    nc.vector.tensor_mul(em, eg[ql][:, jm * P:(jm + 1) * P], mk)
                    emask.append(em)
                # U[j] += Emask[qchunk]^T V[qchunk] for valid blocks (qc <= 2j+1)
                for j in range(() if "av" in _ABLATE else range(2 * g, NJ)) if False else (range(0) if "av" in _ABLATE else range(2 * g, NJ)):
                    hi = min(NQL - 1, 2 * j + 1 - 4 * g)
                    for dv in range(2):
                        ps = psum.tile([P, 512], f32, tag="ps_av", name="ps_av", bufs=1)
                        for ql in range(hi + 1):
                            qc = g * NQL + ql
                            lhs = emask[ql] if j == qc // 2 else \
                                eg[ql][:, j * P:(j + 1) * P]
                            mm(ps, lhs,
                               vg[ql][:, dv * 512:(dv + 1) * 512],
                               start=(ql == 0), stop=(ql == hi))
                        sl = u[j][:, dv * 512:(dv + 1) * 512]
                        if g == 0:
                            nc.vector.tensor_copy(sl, ps)
                        else:
                            nc.vector.tensor_add(sl, sl, ps)
                        if g == min(NG - 1, (2 * j + 1) // NQL):
                            # last contribution to u[j]: ship it now so the
                            # output DMA overlaps the remaining groups
                            dst = out[j * P:(j + 1) * P, dv * 512:(dv + 1) * 512]
                            if accum:
                                nc.gpsimd.dma_start(dst, sl,
                                                    accum_op=mybir.AluOpType.add)
                            else:
                                nc.sync.dma_start(dst, sl)
                # column sums: one psum accumulation chain per kf across ALL
                # 16 q chunks (emitted after AV so the in-order PE stream never
                # stalls waiting for an exp to finish)
                if "cs" not in _ABLATE:
                    for kf in range(2):
                        for ql in range(NQL):
                            qc = g * NQL + ql
                            nc.tensor.matmul(
                                cs_ps[kf], ones_t,
                                eg[ql][:, kf * 512:(kf + 1) * 512],
                                start=(qc == 0), stop=(qc == NG * NQL - 1),
                                skip_group_check=True)

            # ---- epilogue: ship column sums; normalization happens on host ----
            for kf in range(2):
                cs_sb = persist.tile([1, 512], f32, tag=f"cs_sb{kf}",
                                     name=f"cs_sb{kf}")
                nc.vector.tensor_copy(cs_sb, cs_ps[kf])
                dst = cso[:, kf * 512:(kf + 1) * 512]
                if accum:
                    nc.gpsimd.dma_start(dst, cs_sb, accum_op=mybir.AluOpType.add)
                else:
                    nc.sync.dma_start(dst, cs_sb)

    nc.compile()
    return nc


def _get_nc(reps=1, accum=False, use_bf16=False):
    key = ("nc", reps, accum, use_bf16)
    if key not in _cache:
        _cache[key] = _build_module(reps, accum, use_bf16)
    return _cache[key]


def make_in_maps(x, wq, wk, wv, use_bf16=False):
    x = np.asarray(x, np.float32)
    mt = ((np.asarray(wk, np.float64).T @ np.asarray(wq, np.float64))
          / np.sqrt(float(D))).astype(np.float32)
    wvT = np.ascontiguousarray(np.asarray(wv, np.float32).T)
    tri = np.triu(np.ones((P, P), np.float32))
    masks = {
        0: (tri, np.zeros((P, P), np.float32)),          # h=0: diag block, zero block
        1: (np.ones((P, P), np.float32), tri),           # h=1: all-ones block, diag block
    }
    in_maps = []
    for core in range(NCORES):
        b, h = core // 2, core % 2
        xTb = np.ascontiguousarray(x[b].T)               # [D, S]
        cols = np.concatenate(
            [np.arange((2 * j + h) * P, (2 * j + h + 1) * P) for j in range(NJ)])
        xtk = np.ascontiguousarray(xTb[:, cols])         # [D, SK]
        m0, m1 = masks[h]
        m = {
            "xT": xTb, "xtk": xtk, "mt": mt, "wvT": wvT,
            "mask0": m0, "mask1": m1, "onesd": np.ones((P, 1), np.float32),
        }
        if use_bf16:
            import ml_dtypes
            m = {k: v.astype(ml_dtypes.bfloat16) for k, v in m.items()}
        in_maps.append(m)
    return in_maps


def gather(results):
    full = np.empty((B, S, D), np.float32)
    for core in range(NCORES):
        b, h = core // 2, core % 2
        o = results[core]["out"] / results[core]["cso"][0][:, None]
        for j in range(NJ):
            full[b, (2 * j + h) * P:(2 * j + h + 1) * P, :] = \
                o[j * P:(j + 1) * P, :]
    return full


def kernel(x, wq, wk, wv):
    from concourse.bass_utils import run_bass_kernel_spmd
    nc = _get_nc()
    in_maps = make_in_maps(x, wq, wk, wv)
    res = run_bass_kernel_spmd(nc, in_maps, core_ids=list(range(NCORES)))
    return gather(res.results)



# revision 8
# speedup vs baseline: 1.0057x; 1.0057x over previous
"""Trainium2 Bass kernel for a single-head causal self-attention variant.

Reference semantics (B=4, S=2048, D=1024):
    q = x @ wq.T ; k = x @ wk.T ; v = x @ wv.T
    scores = q @ k.T / sqrt(D)          # [B, S, S]
    a = softmax(scores, axis=-2)        # softmax over the QUERY axis, per key column
    a = triu(a)                         # keep q <= k, applied AFTER softmax
    out = a.T @ v                       # out row k = sum_{q<=k} a[q,k] * v[q]

Key algebraic folds (single head):
  * scores = x @ (wq.T @ wk) @ x.T, so wq/wk fold into one matrix
    MT = (wk.T @ wq)/sqrt(D) on the host -> no Q projection on device.
  * softmax needs no max subtraction (scores are O(1) by construction);
    the normalization divide happens on the host: out[k] = U[k] / colsum[k].
  * out = Emask^T @ (x @ wv.T) is reassociated as (Emask^T @ x) @ wv.T:
    the wv projection then acts on the core's SK=1024 output rows instead of
    all S=2048 value rows, which deletes the whole V projection (it was also
    computed redundantly by both cores of a batch). Costs 64 PE transposes
    of the U^T intermediate (bf16, cheap) to feed the final matmul.

Sharding (8 cores): core = (batch b = core//2, half h = core%2). Each core owns
the interleaved local k-chunks kc_global = 2j+h, j=0..7 (128 columns each);
consecutive-pair interleaving is optimal under a uniform SPMD program (per-j
chain length is the max over the core pair). Softmax denominators need all
2048 q per column, so each core computes scores/exp for all q in its 1024
columns; the U accumulation skips blocks the causal mask zeroes entirely, and
the h=0 core's qc=2j+1 block is zeroed via a data mask (mask1=0) so the
program stays uniform.

All matmuls run in bf16 (1 row/cycle on the PE, cheap weight loads; fp32r
pays 4 cycles/row under 256-wide moving dims and ~2x weight-load cost).
E lives in SBUF for the whole kernel (32KB/partition bf16) so each U[j] is a
single PSUM accumulation chain - no SBUF f32 accumulators or tensor_adds.
The device returns unnormalized U @ wv.T plus the column sums (cso); the
O(S*D) divide happens in gather() on the host, and each finished 128-row
output block DMAs out mid-kernel. End-to-end rel-l2 error ~2e-3 vs the fp32
reference (bf16 rounding; budget is 2e-2).
"""

import numpy as np

B, S, D = 4, 2048, 1024
P = 128
SK = 1024          # k columns per core
KD = D // P        # 8 contraction chunks
NJ = SK // P       # 8 local k chunks
NQ = S // P        # 16 q chunks
NG = 4             # q groups of 512
NQL = 4            # 128-row q chunks per group
NCORES = 8

_cache = {}
_ABLATE = set()  # test-only: {"phasek", "e", "ux", "f", "cs"}


def _build_module(reps=1, accum=False):
    import concourse.bacc as bacc
    import concourse.tile as tile
    from concourse import mybir
    from concourse.masks import make_identity
    from contextlib import ExitStack

    f32 = mybir.dt.float32
    bf16 = mybir.dt.bfloat16
    Exp = mybir.ActivationFunctionType.Exp
    Add = mybir.AluOpType.add

    nc = bacc.Bacc("TRN2", target_bir_lowering=False, debug=False,
                   num_devices=NCORES)

    xT = nc.dram_tensor("xT", [D, S], bf16, kind="ExternalInput").ap()
    xr = nc.dram_tensor("xr", [S, D], bf16, kind="ExternalInput").ap()
    xtk = nc.dram_tensor("xtk", [D, SK], bf16, kind="ExternalInput").ap()
    mt = nc.dram_tensor("mt", [D, D], bf16, kind="ExternalInput").ap()
    wvT = nc.dram_tensor("wvT", [D, D], bf16, kind="ExternalInput").ap()
    mask0 = nc.dram_tensor("mask0", [P, P], bf16, kind="ExternalInput").ap()
    mask1 = nc.dram_tensor("mask1", [P, P], bf16, kind="ExternalInput").ap()
    onesd = nc.dram_tensor("onesd", [P, 1], bf16, kind="ExternalInput").ap()
    out = nc.dram_tensor("out", [SK, D], f32, kind="ExternalOutput").ap()
    cso = nc.dram_tensor("cso", [1, SK], f32, kind="ExternalOutput").ap()

    def mm(ps, lhsT, rhs, start, stop):
        # interleaved chain pairs share a stationary tile -> always skip the
        # sim's accumulation-group check (hardware groups are per-bank)
        nc.tensor.matmul(ps, lhsT, rhs, start=start, stop=stop,
                         skip_group_check=True)

    with tile.TileContext(nc) as tc:
        for _rep in range(reps):
          with ExitStack() as ctx:
            persist = ctx.enter_context(tc.tile_pool(name="persist", bufs=1))
            psum = ctx.enter_context(tc.tile_pool(name="psum", bufs=2, space="PSUM"))

            ones_t = persist.tile([P, 1], bf16, tag="ones")
            nc.sync.dma_start(ones_t, onesd)
            m0_t = persist.tile([P, P], bf16, tag="m0")
            nc.sync.dma_start(m0_t, mask0)
            m1_t = persist.tile([P, P], bf16, tag="m1")
            nc.sync.dma_start(m1_t, mask1)
            ident = persist.tile([P, P], bf16, tag="ident")
            make_identity(nc, ident)

            wv_t = persist.tile([P, KD, D], bf16, tag="wv")
            for c in range(KD):
                nc.sync.dma_start(wv_t[:, c, :], wvT[c * P:(c + 1) * P, :])

            km_t = persist.tile([P, KD, SK], bf16, tag="km")
            e_t = persist.tile([P, NQ, SK], bf16, tag="e")    # exp(scores), unmasked
            xr_t = persist.tile([P, NQ, D], bf16, tag="xr")   # x row-chunks [q, dx]
            em_t = [persist.tile([P, P], bf16, tag=f"em{qc}", name=f"em{qc}")
                    for qc in range(NQ)]

            # ---- phase K: KM[dq, k] = sum_dk M[dq,dk] * x_k^T[dk, k] ----
            with tc.tile_pool(name="pk", bufs=1) as pk:
                mt_t = pk.tile([P, KD, D], bf16, tag="mt")
                xtk_t = pk.tile([P, KD, SK], bf16, tag="xtk")
                for c in range(KD):
                    nc.sync.dma_start(mt_t[:, c, :], mt[c * P:(c + 1) * P, :])
                    nc.sync.dma_start(xtk_t[:, c, :], xtk[c * P:(c + 1) * P, :])
                for dq in range(0 if "phasek" in _ABLATE else KD):
                    pss = [psum.tile([P, 512], f32, tag="ps_mm",
                                     name=f"ps_km{kf}", bufs=4) for kf in range(2)]
                    for c in range(KD):
                        for kf in range(2):
                            mm(pss[kf], mt_t[:, c, dq * P:(dq + 1) * P],
                               xtk_t[:, c, kf * 512:(kf + 1) * 512],
                               start=(c == 0), stop=(c == KD - 1))
                    for kf in range(2):
                        nc.vector.tensor_copy(
                            km_t[:, dq, kf * 512:(kf + 1) * 512], pss[kf])

            # ---- phase E: E = exp(x_q^T KM) for all q, kept in SBUF ----
            cs_ps = [psum.tile([1, 512], f32, tag=f"ps_cs{kf}", name=f"ps_cs{kf}",
                               bufs=1) for kf in range(2)]

            def emit_cs(g):
                # colsum chains (denominator over ALL q); deferred one group so
                # the in-order PE stream never waits on an in-flight exp
                if "cs" in _ABLATE:
                    return
                for kf in range(2):
                    for ql in range(NQL):
                        qc = g * NQL + ql
                        nc.tensor.matmul(
                            cs_ps[kf], ones_t,
                            e_t[:, qc, kf * 512:(kf + 1) * 512],
                            start=(qc == 0), stop=(qc == NQ - 1),
                            skip_group_check=True)

            qgp = ctx.enter_context(tc.tile_pool(name="qgp", bufs=2))
            for g in range(NG):
                xg = qgp.tile([P, KD, 512], bf16, tag="xg", name=f"xg{g}")
                for c in range(KD):
                    nc.sync.dma_start(
                        xg[:, c, :], xT[c * P:(c + 1) * P, g * 512:(g + 1) * 512])
                for ql in range(NQL):
                    qc = g * NQL + ql
                    nc.sync.dma_start(xr_t[:, qc, :], xr[qc * P:(qc + 1) * P, :])
                    if "e" not in _ABLATE:
                        pss = [psum.tile([P, 512], f32, tag="ps_mm",
                                         name=f"ps_e{kf}", bufs=4) for kf in range(2)]
                        for c in range(KD):
                            for kf in range(2):
                                mm(pss[kf], xg[:, c, ql * P:(ql + 1) * P],
                                   km_t[:, c, kf * 512:(kf + 1) * 512],
                                   start=(c == 0), stop=(c == KD - 1))
                        for kf in range(2):
                            nc.scalar.activation(
                                e_t[:, qc, kf * 512:(kf + 1) * 512], pss[kf], Exp)
                    # causal boundary: qc=2j   -> mask0 (h=0: triu, h=1: ones)
                    #                  qc=2j+1 -> mask1 (h=0: zero, h=1: triu)
                    jm = qc // 2
                    mk = m0_t if qc % 2 == 0 else m1_t
                    nc.vector.tensor_mul(em_t[qc], e_t[:, qc, jm * P:(jm + 1) * P], mk)
                if g >= 1:
                    emit_cs(g - 1)

            # ---- phase U: U[j] = Emask^T x (one PSUM chain), transpose,
            #      out[j] = U[j]^T^T ... = (U^T)^T wv^T via uxT chains ----
            wp = ctx.enter_context(tc.tile_pool(name="wp", bufs=2))

            def emit_ux(j):
                hi = 2 * j + 1
                u_sb = wp.tile([P, D], bf16, tag="usb", name=f"usb{j}", bufs=2)
                pss = [psum.tile([P, 512], f32, tag="ps_mm",
                                 name=f"ps_ux{dv}", bufs=4) for dv in range(2)]
                for qc in range(hi + 1):
                    lhs = em_t[qc] if qc // 2 == j else \
                        e_t[:, qc, j * P:(j + 1) * P]
                    for dv in range(2):
                        mm(pss[dv], lhs, xr_t[:, qc, dv * 512:(dv + 1) * 512],
                           start=(qc == 0), stop=(qc == hi))
                for dv in range(2):
                    nc.scalar.copy(u_sb[:, dv * 512:(dv + 1) * 512], pss[dv])
                return u_sb

            def emit_t(j, u_sb):
                # U[j] is [k, dx]; the final matmul needs U^T [dx, k] as lhsT
                uxT = wp.tile([P, KD, P], bf16, tag="uxT", name=f"uxT{j}", bufs=2)
                for chalf in range(2):
                    ps_t = psum.tile([P, KD // 2, P], bf16, tag="ps_t",
                                     name=f"ps_t{j}_{chalf}", bufs=2)
                    for ci in range(KD // 2):
                        c = chalf * (KD // 2) + ci
                        nc.tensor.transpose(ps_t[:, ci, :],
                                            u_sb[:, c * P:(c + 1) * P], ident)
                    nc.vector.tensor_copy(
                        uxT[:, chalf * (KD // 2):(chalf + 1) * (KD // 2), :], ps_t)
                return uxT

            def emit_f(j, uxT):
                osb = wp.tile([P, D], f32, tag="osb", name=f"osb{j}", bufs=2)
                pss = [psum.tile([P, 512], f32, tag="ps_mm",
                                 name=f"ps_f{dv}", bufs=4) for dv in range(2)]
                for c in range(KD):
                    for dv in range(2):
                        mm(pss[dv], uxT[:, c, :], wv_t[:, c, dv * 512:(dv + 1) * 512],
                           start=(c == 0), stop=(c == KD - 1))
                for dv in range(2):
                    sl = osb[:, dv * 512:(dv + 1) * 512]
                    nc.vector.tensor_copy(sl, pss[dv])
                    dst = out[j * P:(j + 1) * P, dv * 512:(dv + 1) * 512]
                    if accum:
                        nc.gpsimd.dma_start(dst, sl, accum_op=Add)
                    else:
                        nc.sync.dma_start(dst, sl)

            usb, uxt = {}, {}
            for j in range(NJ):
                if "ux" not in _ABLATE:
                    usb[j] = emit_ux(j)
                if j == 0:
                    emit_cs(NG - 1)
                if "ux" in _ABLATE or "f" in _ABLATE:
                    continue
                if j >= 1:
                    uxt[j - 1] = emit_t(j - 1, usb[j - 1])
                if j >= 2:
                    emit_f(j - 2, uxt[j - 2])
            if "ux" not in _ABLATE and "f" not in _ABLATE:
                uxt[NJ - 1] = emit_t(NJ - 1, usb[NJ - 1])
                emit_f(NJ - 2, uxt[NJ - 2])
                emit_f(NJ - 1, uxt[NJ - 1])

            # ---- epilogue: ship column sums; normalization happens on host ----
            for kf in range(2):
                cs_sb = persist.tile([1, 512], f32, tag=f"cs_sb{kf}",
                                     name=f"cs_sb{kf}")
                nc.vector.tensor_copy(cs_sb, cs_ps[kf])
                dst = cso[:, kf * 512:(kf + 1) * 512]
                if accum:
                    nc.gpsimd.dma_start(dst, cs_sb, accum_op=Add)
                else:
                    nc.sync.dma_start(dst, cs_sb)

    import os
    if os.environ.get("KERNEL_NO_DEDUP") != "1":
        _dedup_ldweights(nc, mybir)
    nc.compile()
    return nc


def _dedup_ldweights(nc, mybir):
    """Drop an InstLdweights whose weights AP matches the immediately
    preceding PE weight load (bf16 matmuls don't self-load, so the PE array
    still holds those weights). Waits/updates of a dropped load move to the
    next PE instruction; generate_event_semaphores later splits multi-waits.
    Any non-Ldweights PE instruction other than a plain matmul (transpose
    loads its input as stationary) resets the tracked state."""
    removed = 0
    for blk in nc.main_func.blocks:
        last_key = None
        keep = []
        pending = None
        for inst in blk.instructions:
            if isinstance(inst, mybir.InstLdweights):
                key = (repr(inst.ins[0]), bool(inst.is_transpose),
                       inst.perf_mode)
                if key == last_key and not inst.is_transpose:
                    si = inst.sync_info
                    if si is not None and (si.on_wait or si.on_update):
                        pending = si
                    removed += 1
                    continue
                last_key = key
            elif isinstance(inst, mybir.InstMatmult):
                if inst.is_transpose:
                    last_key = None
            elif getattr(inst, "engine", None) == nc.tensor.engine:
                last_key = None
            if pending is not None and getattr(inst, "engine", None) == \
                    nc.tensor.engine:
                si = inst.sync_info
                if si is None:
                    inst.sync_info = pending
                else:
                    si.on_wait.extend(pending.on_wait)
                    si.on_update.extend(pending.on_update)
                pending = None
            keep.append(inst)
        assert pending is None, "dangling sync_info from dropped ldweights"
        blk.instructions[:] = keep
    return removed


def _get_nc(reps=1, accum=False):
    key = ("nc", reps, accum)
    if key not in _cache:
        _cache[key] = _build_module(reps, accum)
    return _cache[key]


def make_in_maps(x, wq, wk, wv):
    import ml_dtypes
    bf = ml_dtypes.bfloat16
    x = np.asarray(x, np.float32)
    mt = ((np.asarray(wk, np.float64).T @ np.asarray(wq, np.float64))
          / np.sqrt(float(D))).astype(bf)
    wvTb = np.ascontiguousarray(np.asarray(wv, np.float32).T).astype(bf)
    tri = np.triu(np.ones((P, P), np.float32)).astype(bf)
    ones = np.ones((P, P), bf)
    zeros = np.zeros((P, P), bf)
    onesd = np.ones((P, 1), bf)
    masks = {0: (tri, zeros), 1: (ones, tri)}
    in_maps = []
    xb_bf, xTb_bf = {}, {}
    for b in range(B):
        xb_bf[b] = x[b].astype(bf)
        xTb_bf[b] = np.ascontiguousarray(x[b].T).astype(bf)
    for core in range(NCORES):
        b, h = core // 2, core % 2
        cols = np.concatenate(
            [np.arange((2 * j + h) * P, (2 * j + h + 1) * P) for j in range(NJ)])
        xtk = np.ascontiguousarray(xTb_bf[b][:, cols])
        m0, m1 = masks[h]
        in_maps.append({
            "xT": xTb_bf[b], "xr": xb_bf[b], "xtk": xtk, "mt": mt,
            "wvT": wvTb, "mask0": m0, "mask1": m1, "onesd": onesd,
        })
    return in_maps


def gather(results):
    full = np.empty((B, S, D), np.float32)
    for core in range(NCORES):
        b, h = core // 2, core % 2
        o = results[core]["out"] / results[core]["cso"][0][:, None]
        for j in range(NJ):
            full[b, (2 * j + h) * P:(2 * j + h + 1) * P, :] = \
                o[j * P:(j + 1) * P, :]
    return full


def kernel(x, wq, wk, wv):
    from concourse.bass_utils import run_bass_kernel_spmd
    nc = _get_nc()
    in_maps = make_in_maps(x, wq, wk, wv)
    res = run_bass_kernel_spmd(nc, in_maps, core_ids=list(range(NCORES)))
    return gather(res.results)


# revision 22
# speedup vs baseline: 1.1100x; 1.1038x over previous
"""Trainium2 Bass kernel for a single-head causal self-attention variant.

Reference semantics (B=4, S=2048, D=1024):
    q = x @ wq.T ; k = x @ wk.T ; v = x @ wv.T
    scores = q @ k.T / sqrt(D)          # [B, S, S]
    a = softmax(scores, axis=-2)        # softmax over the QUERY axis, per key column
    a = triu(a)                         # keep q <= k, applied AFTER softmax
    out = a.T @ v                       # out row k = sum_{q<=k} a[q,k] * v[q]

Key algebraic folds (single head):
  * scores = x @ (wq.T @ wk) @ x.T, so wq/wk fold into one matrix
    MT = (wk.T @ wq)/sqrt(D) on the host -> no Q projection on device.
  * softmax needs no max subtraction (scores are O(1) by construction);
    the normalization divide happens on the host: out[k] = U[k] / colsum[k].
  * out = Emask^T @ (x @ wv.T) is reassociated as (Emask^T @ x) @ wv.T:
    the wv projection then acts on the core's SK=1024 output rows instead of
    all S=2048 value rows, which deletes the whole V projection (it was also
    computed redundantly by both cores of a batch). Costs 64 PE transposes
    of the U^T intermediate (bf16, cheap) to feed the final matmul.

Sharding (8 cores): core = (batch b = core//2, half h = core%2). Each core owns
the interleaved local k-chunks kc_global = 2j+h, j=0..7 (128 columns each);
consecutive-pair interleaving is optimal under a uniform SPMD program (per-j
chain length is the max over the core pair). Softmax denominators need all
2048 q per column, so each core computes scores/exp for all q in its 1024
columns; the U accumulation skips blocks the causal mask zeroes entirely, and
the h=0 core's qc=2j+1 block is zeroed via a data mask (mask1=0) so the
program stays uniform.

All matmuls run in bf16 (1 row/cycle on the PE, cheap weight loads; fp32r
pays 4 cycles/row under 256-wide moving dims and ~2x weight-load cost).
E lives in SBUF for the whole kernel (32KB/partition bf16) so each U[j] is a
single PSUM accumulation chain - no SBUF f32 accumulators or tensor_adds.
The device returns unnormalized U @ wv.T plus the column sums (cso); the
O(S*D) divide happens in gather() on the host, and each finished 128-row
output block DMAs out mid-kernel. End-to-end rel-l2 error ~2e-3 vs the fp32
reference (bf16 rounding; budget is 2e-2).
"""

import numpy as np

B, S, D = 4, 2048, 1024
P = 128
SK = 1024          # k columns per core
KD = D // P        # 8 contraction chunks
NJ = SK // P       # 8 local k chunks
NQ = S // P        # 16 q chunks
NG = 4             # q groups of 512
NQL = 4            # 128-row q chunks per group
NCORES = 8

_cache = {}
_ABLATE = set()  # test-only: {"phasek", "e", "ux", "f", "cs"}


def _build_module(reps=1, accum=False):
    import concourse.bacc as bacc
    import concourse.tile as tile
    from concourse import mybir
    from concourse.masks import make_identity
    from contextlib import ExitStack

    f32 = mybir.dt.float32
    bf16 = mybir.dt.bfloat16
    Exp = mybir.ActivationFunctionType.Exp
    Add = mybir.AluOpType.add

    nc = bacc.Bacc("TRN2", target_bir_lowering=False, debug=False,
                   num_devices=NCORES)

    xT = nc.dram_tensor("xT", [D, S], bf16, kind="ExternalInput").ap()
    xr = nc.dram_tensor("xr", [S, D], bf16, kind="ExternalInput").ap()
    xtk = nc.dram_tensor("xtk", [D, SK], bf16, kind="ExternalInput").ap()
    mt = nc.dram_tensor("mt", [D, D], bf16, kind="ExternalInput").ap()
    wvT = nc.dram_tensor("wvT", [D, D], bf16, kind="ExternalInput").ap()
    mask0 = nc.dram_tensor("mask0", [P, P], bf16, kind="ExternalInput").ap()
    mask1 = nc.dram_tensor("mask1", [P, P], bf16, kind="ExternalInput").ap()
    onesd = nc.dram_tensor("onesd", [P, 1], bf16, kind="ExternalInput").ap()
    out = nc.dram_tensor("out", [SK, D], f32, kind="ExternalOutput").ap()
    cso = nc.dram_tensor("cso", [1, SK], f32, kind="ExternalOutput").ap()

    def mm(ps, lhsT, rhs, start, stop):
        # interleaved chain pairs share a stationary tile -> always skip the
        # sim's accumulation-group check (hardware groups are per-bank)
        nc.tensor.matmul(ps, lhsT, rhs, start=start, stop=stop,
                         skip_group_check=True)

    with tile.TileContext(nc) as tc:
        for _rep in range(reps):
          with ExitStack() as ctx:
            persist = ctx.enter_context(tc.tile_pool(name="persist", bufs=1))
            psum = ctx.enter_context(tc.tile_pool(name="psum", bufs=2, space="PSUM"))

            ones_t = persist.tile([P, 1], bf16, tag="ones")
            nc.sync.dma_start(ones_t, onesd)
            m0_t = persist.tile([P, P], bf16, tag="m0")
            nc.sync.dma_start(m0_t, mask0)
            m1_t = persist.tile([P, P], bf16, tag="m1")
            nc.sync.dma_start(m1_t, mask1)
            ident = persist.tile([P, P], bf16, tag="ident")
            make_identity(nc, ident)

            wv_t = persist.tile([P, KD, D], bf16, tag="wv")
            km_t = persist.tile([P, KD, SK], bf16, tag="km")
            e_t = persist.tile([P, NQ, SK], bf16, tag="e")    # exp(scores), unmasked
            xr_t = persist.tile([P, NQ, D], bf16, tag="xr")   # x row-chunks [q, dx]
            em_t = [persist.tile([P, P], bf16, tag=f"em{qc}", name=f"em{qc}")
                    for qc in range(NQ)]

            # ---- phase K: KM[dq, k] = sum_dk M[dq,dk] * x_k^T[dk, k] ----
            qgp = ctx.enter_context(tc.tile_pool(name="qgp", bufs=2))
            xg_tiles = {}
            with tc.tile_pool(name="pk", bufs=1) as pk:
                mt_t = pk.tile([P, KD, D], bf16, tag="mt")
                xtk_t = pk.tile([P, KD, SK], bf16, tag="xtk")
                for c in range(KD):
                    nc.sync.dma_start(mt_t[:, c, :], mt[c * P:(c + 1) * P, :])
                    nc.sync.dma_start(xtk_t[:, c, :], xtk[c * P:(c + 1) * P, :])
                # pre-issue the E-phase inputs the first groups need, so the
                # PE never stalls at the K->E boundary; wv (only needed by
                # phase U) queues after them
                for g0 in range(2):
                    xg_tiles[g0] = qgp.tile([P, KD, 512], bf16, tag="xg",
                                            name=f"xg{g0}")
                    for c in range(KD):
                        nc.sync.dma_start(
                            xg_tiles[g0][:, c, :],
                            xT[c * P:(c + 1) * P, g0 * 512:(g0 + 1) * 512])
                for qc in range(2 * NQL):
                    nc.sync.dma_start(xr_t[:, qc, :], xr[qc * P:(qc + 1) * P, :])
                for c in range(KD):
                    nc.sync.dma_start(wv_t[:, c, :], wvT[c * P:(c + 1) * P, :])
                for dq in range(0 if "phasek" in _ABLATE else KD):
                    for kf in range(2):
                        ps = psum.tile([P, 512], f32, tag="ps_mm", name="ps_km", bufs=4)
                        for c in range(KD):
                            mm(ps, mt_t[:, c, dq * P:(dq + 1) * P],
                               xtk_t[:, c, kf * 512:(kf + 1) * 512],
                               start=(c == 0), stop=(c == KD - 1))
                        nc.vector.tensor_copy(km_t[:, dq, kf * 512:(kf + 1) * 512], ps)

            # ---- phase E: E = exp(x_q^T KM) for all q, kept in SBUF ----
            cs_ps = [psum.tile([1, 512], f32, tag=f"ps_cs{kf}", name=f"ps_cs{kf}",
                               bufs=1) for kf in range(2)]

            def emit_cs(g):
                # colsum chains (denominator over ALL q); deferred one group so
                # the in-order PE stream never waits on an in-flight exp
                if "cs" in _ABLATE:
                    return
                for kf in range(2):
                    for ql in range(NQL):
                        qc = g * NQL + ql
                        nc.tensor.matmul(
                            cs_ps[kf], ones_t,
                            e_t[:, qc, kf * 512:(kf + 1) * 512],
                            start=(qc == 0), stop=(qc == NQ - 1),
                            skip_group_check=True)

            for g in range(NG):
                if g in xg_tiles:
                    xg = xg_tiles[g]
                else:
                    xg = qgp.tile([P, KD, 512], bf16, tag="xg", name=f"xg{g}")
                    for c in range(KD):
                        nc.sync.dma_start(
                            xg[:, c, :], xT[c * P:(c + 1) * P, g * 512:(g + 1) * 512])
                for ql in range(NQL):
                    qc = g * NQL + ql
                    if qc >= 2 * NQL:
                        nc.sync.dma_start(xr_t[:, qc, :], xr[qc * P:(qc + 1) * P, :])
                    if "e" not in _ABLATE:
                        for kf in range(2):
                            ps = psum.tile([P, 512], f32, tag="ps_mm", name="ps_e", bufs=4)
                            for c in range(KD):
                                mm(ps, xg[:, c, ql * P:(ql + 1) * P],
                                   km_t[:, c, kf * 512:(kf + 1) * 512],
                                   start=(c == 0), stop=(c == KD - 1))
                            nc.scalar.activation(e_t[:, qc, kf * 512:(kf + 1) * 512], ps, Exp)
                    # causal boundary: qc=2j   -> mask0 (h=0: triu, h=1: ones)
                    #                  qc=2j+1 -> mask1 (h=0: zero, h=1: triu)
                    jm = qc // 2
                    mk = m0_t if qc % 2 == 0 else m1_t
                    nc.vector.tensor_mul(em_t[qc], e_t[:, qc, jm * P:(jm + 1) * P], mk)
                if g >= 1:
                    emit_cs(g - 1)

            # ---- phase U: U[j] = Emask^T x (one PSUM chain), transpose,
            #      out[j] = U[j]^T^T ... = (U^T)^T wv^T via uxT chains ----
            wp = ctx.enter_context(tc.tile_pool(name="wp", bufs=2))

            def emit_ux(j):
                hi = 2 * j + 1
                u_sb = wp.tile([P, D], bf16, tag="usb", name=f"usb{j}", bufs=2)
                for dv in range(2):
                    ps = psum.tile([P, 512], f32, tag="ps_mm", name="ps_ux", bufs=4)
                    for qc in range(hi + 1):
                        lhs = em_t[qc] if qc // 2 == j else \
                            e_t[:, qc, j * P:(j + 1) * P]
                        mm(ps, lhs, xr_t[:, qc, dv * 512:(dv + 1) * 512],
                           start=(qc == 0), stop=(qc == hi))
                    nc.scalar.copy(u_sb[:, dv * 512:(dv + 1) * 512], ps)
                return u_sb

            def emit_t(j, u_sb):
                # U[j] is [k, dx]; the final matmul needs U^T [dx, k] as lhsT
                # (XBAR DMA transpose measured wrong-layout AND ~25us slower,
                # so PE transposes it)
                uxT = wp.tile([P, KD, P], bf16, tag="uxT", name=f"uxT{j}", bufs=2)
                for chalf in range(2):
                    ps_t = psum.tile([P, KD // 2, P], bf16, tag="ps_t",
                                     name=f"ps_t{j}_{chalf}", bufs=2)
                    for ci in range(KD // 2):
                        c = chalf * (KD // 2) + ci
                        nc.tensor.transpose(ps_t[:, ci, :],
                                            u_sb[:, c * P:(c + 1) * P], ident)
                    nc.vector.tensor_copy(
                        uxT[:, chalf * (KD // 2):(chalf + 1) * (KD // 2), :], ps_t)
                return uxT

            def emit_f(j, uxT):
                osb = wp.tile([P, D], f32, tag="osb", name=f"osb{j}", bufs=2)
                for dv in range(2):
                    ps = psum.tile([P, 512], f32, tag="ps_mm", name="ps_f", bufs=4)
                    for c in range(KD):
                        mm(ps, uxT[:, c, :], wv_t[:, c, dv * 512:(dv + 1) * 512],
                           start=(c == 0), stop=(c == KD - 1))
                    sl = osb[:, dv * 512:(dv + 1) * 512]
                    nc.vector.tensor_copy(sl, ps)
                    dst = out[j * P:(j + 1) * P, dv * 512:(dv + 1) * 512]
                    if accum:
                        nc.gpsimd.dma_start(dst, sl, accum_op=Add)
                    else:
                        nc.sync.dma_start(dst, sl)

            usb, uxt = {}, {}
            for j in range(NJ):
                if "ux" not in _ABLATE:
                    usb[j] = emit_ux(j)
                if j == 0:
                    emit_cs(NG - 1)
                if "ux" in _ABLATE or "f" in _ABLATE:
                    continue
                if j >= 1:
                    uxt[j - 1] = emit_t(j - 1, usb[j - 1])
                if j >= 2:
                    emit_f(j - 2, uxt[j - 2])
            if "ux" not in _ABLATE and "f" not in _ABLATE:
                uxt[NJ - 1] = emit_t(NJ - 1, usb[NJ - 1])
                emit_f(NJ - 2, uxt[NJ - 2])
                emit_f(NJ - 1, uxt[NJ - 1])

            # ---- epilogue: ship column sums; normalization happens on host ----
            for kf in range(2):
                cs_sb = persist.tile([1, 512], f32, tag=f"cs_sb{kf}",
                                     name=f"cs_sb{kf}")
                nc.vector.tensor_copy(cs_sb, cs_ps[kf])
                dst = cso[:, kf * 512:(kf + 1) * 512]
                if accum:
                    nc.gpsimd.dma_start(dst, cs_sb, accum_op=Add)
                else:
                    nc.sync.dma_start(dst, cs_sb)

    import os
    if os.environ.get("KERNEL_DEDUP") == "1":
        # measured: no gain (ldweights overlap with matmuls via the shadow
        # bank) and the chain interleave it needs costs ~11us; keep off
        _dedup_ldweights(nc, mybir)
    nc.compile()
    return nc


def _dedup_ldweights(nc, mybir):
    """Drop an InstLdweights whose weights AP matches the immediately
    preceding PE weight load (bf16 matmuls don't self-load, so the PE array
    still holds those weights). Waits/updates of a dropped load move to the
    next PE instruction; generate_event_semaphores later splits multi-waits.
    Any non-Ldweights PE instruction other than a plain matmul (transpose
    loads its input as stationary) resets the tracked state."""
    removed = 0
    for blk in nc.main_func.blocks:
        last_key = None
        keep = []
        pending = None
        for inst in blk.instructions:
            if isinstance(inst, mybir.InstLdweights):
                key = (repr(inst.ins[0]), bool(inst.is_transpose),
                       inst.perf_mode)
                if key == last_key and not inst.is_transpose:
                    si = inst.sync_info
                    if si is not None and (si.on_wait or si.on_update):
                        pending = si
                    removed += 1
                    continue
                last_key = key
            elif isinstance(inst, mybir.InstMatmult):
                if inst.is_transpose:
                    last_key = None
            elif getattr(inst, "engine", None) == nc.tensor.engine:
                last_key = None
            if pending is not None and getattr(inst, "engine", None) == \
                    nc.tensor.engine:
                si = inst.sync_info
                if si is None:
                    inst.sync_info = pending
                else:
                    si.on_wait.extend(pending.on_wait)
                    si.on_update.extend(pending.on_update)
                pending = None
            keep.append(inst)
        assert pending is None, "dangling sync_info from dropped ldweights"
        blk.instructions[:] = keep
    return removed


def _get_nc(reps=1, accum=False):
    key = ("nc", reps, accum)
    if key not in _cache:
        _cache[key] = _build_module(reps, accum)
    return _cache[key]


def make_in_maps(x, wq, wk, wv):
    import ml_dtypes
    bf = ml_dtypes.bfloat16
    x = np.asarray(x, np.float32)
    mt = ((np.asarray(wk, np.float64).T @ np.asarray(wq, np.float64))
          / np.sqrt(float(D))).astype(bf)
    wvTb = np.ascontiguousarray(np.asarray(wv, np.float32).T).astype(bf)
    tri = np.triu(np.ones((P, P), np.float32)).astype(bf)
    ones = np.ones((P, P), bf)
    zeros = np.zeros((P, P), bf)
    onesd = np.ones((P, 1), bf)
    masks = {0: (tri, zeros), 1: (ones, tri)}
    in_maps = []
    xb_bf, xTb_bf = {}, {}
    for b in range(B):
        xb_bf[b] = x[b].astype(bf)
        xTb_bf[b] = np.ascontiguousarray(x[b].T).astype(bf)
    for core in range(NCORES):
        b, h = core // 2, core % 2
        cols = np.concatenate(
            [np.arange((2 * j + h) * P, (2 * j + h + 1) * P) for j in range(NJ)])
        xtk = np.ascontiguousarray(xTb_bf[b][:, cols])
        m0, m1 = masks[h]
        in_maps.append({
            "xT": xTb_bf[b], "xr": xb_bf[b], "xtk": xtk, "mt": mt,
            "wvT": wvTb, "mask0": m0, "mask1": m1, "onesd": onesd,
        })
    return in_maps


def gather(results):
    full = np.empty((B, S, D), np.float32)
    for core in range(NCORES):
        b, h = core // 2, core % 2
        o = results[core]["out"] / results[core]["cso"][0][:, None]
        for j in range(NJ):
            full[b, (2 * j + h) * P:(2 * j + h + 1) * P, :] = \
                o[j * P:(j + 1) * P, :]
    return full


def kernel(x, wq, wk, wv):
    from concourse.bass_utils import run_bass_kernel_spmd
    nc = _get_nc()
    in_maps = make_in_maps(x, wq, wk, wv)
    res = run_bass_kernel_spmd(nc, in_maps, core_ids=list(range(NCORES)))
    return gather(res.results)


# revision 32
# speedup vs baseline: 1.1584x; 1.0436x over previous
"""Trainium2 Bass kernel for a single-head causal self-attention variant.

Reference semantics (B=4, S=2048, D=1024):
    q = x @ wq.T ; k = x @ wk.T ; v = x @ wv.T
    scores = q @ k.T / sqrt(D)          # [B, S, S]
    a = softmax(scores, axis=-2)        # softmax over the QUERY axis, per key column
    a = triu(a)                         # keep q <= k, applied AFTER softmax
    out = a.T @ v                       # out row k = sum_{q<=k} a[q,k] * v[q]

Key algebraic folds (single head):
  * scores = x @ (wq.T @ wk) @ x.T, so wq/wk fold into one matrix
    MT = (wk.T @ wq)/sqrt(D) on the host -> no Q projection on device.
  * softmax needs no max subtraction (scores are O(1) by construction);
    the normalization divide happens on the host: out[k] = U[k] / colsum[k].
  * out = Emask^T @ (x @ wv.T) is reassociated as (Emask^T @ x) @ wv.T:
    the wv projection then acts on the core's SK=1024 output rows instead of
    all S=2048 value rows, which deletes the whole V projection (it was also
    computed redundantly by both cores of a batch). Costs 64 PE transposes
    of the U^T intermediate (bf16, cheap) to feed the final matmul.

Sharding (8 cores): core = (batch b = core//2, half h = core%2). Each core owns
the interleaved local k-chunks kc_global = 2j+h, j=0..7 (128 columns each);
consecutive-pair interleaving is optimal under a uniform SPMD program (per-j
chain length is the max over the core pair). Softmax denominators need all
2048 q per column, so each core computes scores/exp for all q in its 1024
columns; the U accumulation skips blocks the causal mask zeroes entirely, and
the h=0 core's qc=2j+1 block is zeroed via a data mask (mask1=0) so the
program stays uniform.

All matmuls run in bf16 (1 row/cycle on the PE; fp32r pays its weight load
serially, bf16 ldweights overlap via the shadow bank - measured: deduping
ldweights gains nothing and the chain interleave it needs costs ~11us, and
XBAR DMA transpose is ~25us slower than PE transposes, so neither is used).
E lives in SBUF for the whole kernel (32KB/partition bf16) so each U[j] is a
single PSUM accumulation chain - no SBUF f32 accumulators or tensor_adds.
The first E-group and U-phase inputs (xg/xr) are DMA'd during phase K and wv
after them, which removed a ~8us PE stall at the K->E boundary. The device
returns unnormalized U @ wv.T plus the column sums (cso); the O(S*D) divide
happens in gather() on the host, and each finished 128-row output block DMAs
out mid-kernel. Measured: rel-l2 error 3.1e-3 (budget 2e-2), ~190us/core
steady-state vs the ~150us PE row floor (was ~211us for the fp32r V-path
baseline measured the same way).
"""

import numpy as np

B, S, D = 4, 2048, 1024
P = 128
SK = 1024          # k columns per core
KD = D // P        # 8 contraction chunks
NJ = SK // P       # 8 local k chunks
NQ = S // P        # 16 q chunks
NG = 4             # q groups of 512
NQL = 4            # 128-row q chunks per group
NCORES = 8

_cache = {}
_ABLATE = set()  # test-only: {"phasek", "e", "ux", "f", "cs"}


def _build_module(reps=1, accum=False):
    import concourse.bacc as bacc
    import concourse.tile as tile
    from concourse import mybir
    from concourse.masks import make_identity
    from contextlib import ExitStack

    f32 = mybir.dt.float32
    bf16 = mybir.dt.bfloat16
    Exp = mybir.ActivationFunctionType.Exp
    Add = mybir.AluOpType.add

    nc = bacc.Bacc("TRN2", target_bir_lowering=False, debug=False,
                   num_devices=NCORES)

    xT = nc.dram_tensor("xT", [D, S], bf16, kind="ExternalInput").ap()
    xr = nc.dram_tensor("xr", [S, D], bf16, kind="ExternalInput").ap()
    xtk = nc.dram_tensor("xtk", [D, SK], bf16, kind="ExternalInput").ap()
    mt = nc.dram_tensor("mt", [D, D], bf16, kind="ExternalInput").ap()
    wvT = nc.dram_tensor("wvT", [D, D], bf16, kind="ExternalInput").ap()
    mask0 = nc.dram_tensor("mask0", [P, P], bf16, kind="ExternalInput").ap()
    mask1 = nc.dram_tensor("mask1", [P, P], bf16, kind="ExternalInput").ap()
    onesd = nc.dram_tensor("onesd", [P, 1], bf16, kind="ExternalInput").ap()
    out = nc.dram_tensor("out", [SK, D], f32, kind="ExternalOutput").ap()
    # per-partition partial column sums; host finishes the 128-way reduction
    cso = nc.dram_tensor("cso", [P, SK], f32, kind="ExternalOutput").ap()

    def mm(ps, lhsT, rhs, start, stop):
        # interleaved chain pairs share a stationary tile -> always skip the
        # sim's accumulation-group check (hardware groups are per-bank)
        nc.tensor.matmul(ps, lhsT, rhs, start=start, stop=stop,
                         skip_group_check=True)

    with tile.TileContext(nc) as tc:
        for _rep in range(reps):
          with ExitStack() as ctx:
            persist = ctx.enter_context(tc.tile_pool(name="persist", bufs=1))
            psum = ctx.enter_context(tc.tile_pool(name="psum", bufs=2, space="PSUM"))

            m0_t = persist.tile([P, P], bf16, tag="m0")
            nc.sync.dma_start(m0_t, mask0)
            m1_t = persist.tile([P, P], bf16, tag="m1")
            nc.sync.dma_start(m1_t, mask1)
            ident = persist.tile([P, P], bf16, tag="ident")
            make_identity(nc, ident)

            wv_t = persist.tile([P, KD, D], bf16, tag="wv")
            km_t = persist.tile([P, KD, SK], bf16, tag="km")
            e_t = persist.tile([P, NQ, SK], bf16, tag="e")    # exp(scores), unmasked
            xr_t = persist.tile([P, NQ, D], bf16, tag="xr")   # x row-chunks [q, dx]
            em_t = [persist.tile([P, P], bf16, tag=f"em{qc}", name=f"em{qc}")
                    for qc in range(NQ)]

            # ---- phase K: KM[dq, k] = sum_dk M[dq,dk] * x_k^T[dk, k] ----
            qgp = ctx.enter_context(tc.tile_pool(name="qgp", bufs=2))
            xg_tiles = {}
            with tc.tile_pool(name="pk", bufs=1) as pk:
                mt_t = pk.tile([P, KD, D], bf16, tag="mt")
                xtk_t = pk.tile([P, KD, SK], bf16, tag="xtk")
                for c in range(KD):
                    nc.sync.dma_start(mt_t[:, c, :], mt[c * P:(c + 1) * P, :])
                    nc.sync.dma_start(xtk_t[:, c, :], xtk[c * P:(c + 1) * P, :])
                # pre-issue the E-phase inputs the first groups need, so the
                # PE never stalls at the K->E boundary; wv (only needed by
                # phase U) queues after them
                for g0 in range(2):
                    xg_tiles[g0] = qgp.tile([P, KD, 512], bf16, tag="xg",
                                            name=f"xg{g0}")
                    for c in range(KD):
                        nc.sync.dma_start(
                            xg_tiles[g0][:, c, :],
                            xT[c * P:(c + 1) * P, g0 * 512:(g0 + 1) * 512])
                for qc in range(2 * NQL):
                    nc.sync.dma_start(xr_t[:, qc, :], xr[qc * P:(qc + 1) * P, :])
                for c in range(KD):
                    nc.sync.dma_start(wv_t[:, c, :], wvT[c * P:(c + 1) * P, :])
                for dq in range(0 if "phasek" in _ABLATE else KD):
                    for kf in range(2):
                        ps = psum.tile([P, 512], f32, tag="ps_mm", name="ps_km", bufs=4)
                        for c in range(KD):
                            mm(ps, mt_t[:, c, dq * P:(dq + 1) * P],
                               xtk_t[:, c, kf * 512:(kf + 1) * 512],
                               start=(c == 0), stop=(c == KD - 1))
                        nc.vector.tensor_copy(km_t[:, dq, kf * 512:(kf + 1) * 512], ps)

            # ---- phase E: E = exp(x_q^T KM) for all q, kept in SBUF ----
            # softmax denominator: accumulate E over q-chunks on the DVE
            # (f32, exact) instead of 32 ones-matmuls on the PE; the final
            # 128-partition reduction happens on the host in gather()
            csacc = persist.tile([P, SK], f32, tag="csacc")

            for g in range(NG):
                if g in xg_tiles:
                    xg = xg_tiles[g]
                else:
                    xg = qgp.tile([P, KD, 512], bf16, tag="xg", name=f"xg{g}")
                    for c in range(KD):
                        nc.sync.dma_start(
                            xg[:, c, :], xT[c * P:(c + 1) * P, g * 512:(g + 1) * 512])
                for ql in range(NQL):
                    qc = g * NQL + ql
                    if qc >= 2 * NQL:
                        nc.sync.dma_start(xr_t[:, qc, :], xr[qc * P:(qc + 1) * P, :])
                    if "e" not in _ABLATE:
                        for kf in range(2):
                            ps = psum.tile([P, 512], f32, tag="ps_mm", name="ps_e", bufs=4)
                            for c in range(KD):
                                mm(ps, xg[:, c, ql * P:(ql + 1) * P],
                                   km_t[:, c, kf * 512:(kf + 1) * 512],
                                   start=(c == 0), stop=(c == KD - 1))
                            nc.scalar.activation(e_t[:, qc, kf * 512:(kf + 1) * 512], ps, Exp)
                    # causal boundary: qc=2j   -> mask0 (h=0: triu, h=1: ones)
                    #                  qc=2j+1 -> mask1 (h=0: zero, h=1: triu)
                    jm = qc // 2
                    mk = m0_t if qc % 2 == 0 else m1_t
                    nc.vector.tensor_mul(em_t[qc], e_t[:, qc, jm * P:(jm + 1) * P], mk)
                    if "cs" not in _ABLATE:
                        if qc == 0:
                            nc.vector.tensor_copy(csacc, e_t[:, 0, :])
                        else:
                            nc.vector.tensor_add(csacc, csacc, e_t[:, qc, :])

            # partial column sums are final once the E loop ends; ship them
            # now so the DMA overlaps phase U (host reduces+divides)
            if accum:
                nc.gpsimd.dma_start(cso, csacc, accum_op=Add)
            else:
                nc.sync.dma_start(cso, csacc)

            # ---- phase U: U[j] = Emask^T x (one PSUM chain), transpose,
            #      out[j] = U[j]^T^T ... = (U^T)^T wv^T via uxT chains ----
            wp = ctx.enter_context(tc.tile_pool(name="wp", bufs=2))

            def emit_ux(j):
                hi = 2 * j + 1
                u_sb = wp.tile([P, D], bf16, tag="usb", name=f"usb{j}", bufs=2)
                for dv in range(2):
                    ps = psum.tile([P, 512], f32, tag="ps_mm", name="ps_ux", bufs=4)
                    for qc in range(hi + 1):
                        lhs = em_t[qc] if qc // 2 == j else \
                            e_t[:, qc, j * P:(j + 1) * P]
                        mm(ps, lhs, xr_t[:, qc, dv * 512:(dv + 1) * 512],
                           start=(qc == 0), stop=(qc == hi))
                    nc.scalar.copy(u_sb[:, dv * 512:(dv + 1) * 512], ps)
                return u_sb

            def emit_t(j, u_sb):
                # U[j] is [k, dx]; the final matmul needs U^T [dx, k] as lhsT
                # (XBAR DMA transpose measured wrong-layout AND ~25us slower,
                # so PE transposes it)
                uxT = wp.tile([P, KD, P], bf16, tag="uxT", name=f"uxT{j}", bufs=2)
                for chalf in range(2):
                    ps_t = psum.tile([P, KD // 2, P], bf16, tag="ps_t",
                                     name=f"ps_t{j}_{chalf}", bufs=2)
                    for ci in range(KD // 2):
                        c = chalf * (KD // 2) + ci
                        nc.tensor.transpose(ps_t[:, ci, :],
                                            u_sb[:, c * P:(c + 1) * P], ident)
                    nc.vector.tensor_copy(
                        uxT[:, chalf * (KD // 2):(chalf + 1) * (KD // 2), :], ps_t)
                return uxT

            def emit_f(j, uxT):
                osb = wp.tile([P, D], f32, tag="osb", name=f"osb{j}", bufs=2)
                for dv in range(2):
                    ps = psum.tile([P, 512], f32, tag="ps_mm", name="ps_f", bufs=4)
                    for c in range(KD):
                        mm(ps, uxT[:, c, :], wv_t[:, c, dv * 512:(dv + 1) * 512],
                           start=(c == 0), stop=(c == KD - 1))
                    sl = osb[:, dv * 512:(dv + 1) * 512]
                    nc.vector.tensor_copy(sl, ps)
                    dst = out[j * P:(j + 1) * P, dv * 512:(dv + 1) * 512]
                    if accum:
                        nc.gpsimd.dma_start(dst, sl, accum_op=Add)
                    else:
                        nc.sync.dma_start(dst, sl)

            usb, uxt = {}, {}
            for j in range(NJ):
                if "ux" not in _ABLATE:
                    usb[j] = emit_ux(j)
                if "ux" in _ABLATE or "f" in _ABLATE:
                    continue
                if j >= 1:
                    uxt[j - 1] = emit_t(j - 1, usb[j - 1])
                if j >= 2:
                    emit_f(j - 2, uxt[j - 2])
            if "ux" not in _ABLATE and "f" not in _ABLATE:
                uxt[NJ - 1] = emit_t(NJ - 1, usb[NJ - 1])
                emit_f(NJ - 2, uxt[NJ - 2])
                emit_f(NJ - 1, uxt[NJ - 1])



    import os
    if os.environ.get("KERNEL_DEDUP") == "1":
        # measured: no gain (ldweights overlap with matmuls via the shadow
        # bank) and the chain interleave it needs costs ~11us; keep off
        _dedup_ldweights(nc, mybir)
    nc.compile()
    return nc


def _dedup_ldweights(nc, mybir):
    """Drop an InstLdweights whose weights AP matches the immediately
    preceding PE weight load (bf16 matmuls don't self-load, so the PE array
    still holds those weights). Waits/updates of a dropped load move to the
    next PE instruction; generate_event_semaphores later splits multi-waits.
    Any non-Ldweights PE instruction other than a plain matmul (transpose
    loads its input as stationary) resets the tracked state."""
    removed = 0
    for blk in nc.main_func.blocks:
        last_key = None
        keep = []
        pending = None
        for inst in blk.instructions:
            if isinstance(inst, mybir.InstLdweights):
                key = (repr(inst.ins[0]), bool(inst.is_transpose),
                       inst.perf_mode)
                if key == last_key and not inst.is_transpose:
                    si = inst.sync_info
                    if si is not None and (si.on_wait or si.on_update):
                        pending = si
                    removed += 1
                    continue
                last_key = key
            elif isinstance(inst, mybir.InstMatmult):
                if inst.is_transpose:
                    last_key = None
            elif getattr(inst, "engine", None) == nc.tensor.engine:
                last_key = None
            if pending is not None and getattr(inst, "engine", None) == \
                    nc.tensor.engine:
                si = inst.sync_info
                if si is None:
                    inst.sync_info = pending
                else:
                    si.on_wait.extend(pending.on_wait)
                    si.on_update.extend(pending.on_update)
                pending = None
            keep.append(inst)
        assert pending is None, "dangling sync_info from dropped ldweights"
        blk.instructions[:] = keep
    return removed


def _get_nc(reps=1, accum=False):
    key = ("nc", reps, accum)
    if key not in _cache:
        _cache[key] = _build_module(reps, accum)
    return _cache[key]


def make_in_maps(x, wq, wk, wv):
    import ml_dtypes
    bf = ml_dtypes.bfloat16
    x = np.asarray(x, np.float32)
    mt = ((np.asarray(wk, np.float64).T @ np.asarray(wq, np.float64))
          / np.sqrt(float(D))).astype(bf)
    wvTb = np.ascontiguousarray(np.asarray(wv, np.float32).T).astype(bf)
    tri = np.triu(np.ones((P, P), np.float32)).astype(bf)
    ones = np.ones((P, P), bf)
    zeros = np.zeros((P, P), bf)
    onesd = np.ones((P, 1), bf)
    masks = {0: (tri, zeros), 1: (ones, tri)}
    in_maps = []
    xb_bf, xTb_bf = {}, {}
    for b in range(B):
        xb_bf[b] = x[b].astype(bf)
        xTb_bf[b] = np.ascontiguousarray(x[b].T).astype(bf)
    for core in range(NCORES):
        b, h = core // 2, core % 2
        cols = np.concatenate(
            [np.arange((2 * j + h) * P, (2 * j + h + 1) * P) for j in range(NJ)])
        xtk = np.ascontiguousarray(xTb_bf[b][:, cols])
        m0, m1 = masks[h]
        in_maps.append({
            "xT": xTb_bf[b], "xr": xb_bf[b], "xtk": xtk, "mt": mt,
            "wvT": wvTb, "mask0": m0, "mask1": m1, "onesd": onesd,
        })
    return in_maps


def gather(results):
    full = np.empty((B, S, D), np.float32)
    for core in range(NCORES):
        b, h = core // 2, core % 2
        o = results[core]["out"] / \
            results[core]["cso"].sum(axis=0, dtype=np.float64)[:, None]
        for j in range(NJ):
            full[b, (2 * j + h) * P:(2 * j + h + 1) * P, :] = \
                o[j * P:(j + 1) * P, :]
    return full


def kernel(x, wq, wk, wv):
    from concourse.bass_utils import run_bass_kernel_spmd
    nc = _get_nc()
    in_maps = make_in_maps(x, wq, wk, wv)
    res = run_bass_kernel_spmd(nc, in_maps, core_ids=list(range(NCORES)))
    return gather(res.results)
